# revision 33
# baseline (speedup 1.0000x reference)
"""Trainium2 Bass kernel for nn_BagModel_3d (segment_reduce).

Computation (per bag b):
  out[b] = (1/n_b) * sum_{i < n_b} relu(x[b, i, :] @ W1 + b1) @ W2 + b2

Strategy (v3 -- instances-on-partitions dataflow, fp8 DoubleRow z):

- Data-parallel over bags: LPT assigns exactly 32 bags per core
  (descending n, so small bags land at the tail of each core's column
  space). Valid instance columns are laid out contiguously and split
  into 128-column CHUNKs.
- Per chunk, the x block is the matmul STATIONARY operand and W1
  streams, so z lands as [cols(part), dh(free)] in PSUM. Chunks whose
  bags all have n >= SMALL_N use fp8-e4m3 x/W1 with perf_mode=DoubleRow
  (contraction 256 in ONE matmul; quantization noise averages down as
  1/sqrt(n) in the bag mean); chunks touching smaller bags use bf16
  with two matmuls. Two chunks share one PSUM bank; one relu per bank
  ([128, 512], ScalarE activation / VectorE tensor_scalar alternating)
  produces h in bf16 -- no per-bag drains, no on-device casts.
- The ragged per-bag sum is a 0/1 selector matmul: praw[bag, dh] +=
  sel_c^T @ h_c, accumulated in PSUM across all chunks; bag boundaries
  are data, padding columns contribute exactly 0, and chunks are
  striped over the 4 PE column-groups (tile_position) so 4 selector
  matmuls run concurrently in the array.
- Finalization: o4 = sum_dh praw*W2 (one fused DVE op with accum),
  strip-sum via a tiny matmul, then out = o4/n + b2.
- DMA: each HWDGE ring serializes its ops with a ~2.2us completion
  tax, so ops are few and large: op A (sync) carries w1+first chunks
  in ONE op to minimize PE start latency; the sel matrix is split so
  its head rides early on the scalar ring; slow-start SWDGE (gpsimd)
  carries late-needed data (aux consts, bf16 tail chunks).
- b1 (zero in this workload) is handled generally via a rank-1 k=1
  matmul added into z before relu, emitted only when b1 != 0.
"""
import sys
import numpy as np

sys.path.insert(0, '/opt/trn_rl_repo')

import ml_dtypes

B, N_MAX, D_IN, D_H = 256, 512, 256, 256
N_CORES = 8
BAGS = B // N_CORES          # 32 bags per core
CHUNK = 128                  # instance columns per matmul chunk
COL_TILE = True              # stripe selector matmuls over PE col-groups
SEL_AT_END = True            # emit all selector MMs after the z stream
USE_FP8 = True               # fp8 DoubleRow z for big-bag chunks
SMALL_N = 16                 # bags below this stay bf16
A_CH = 8                     # x chunks bundled with w1+sel-head in op A
SELH = 16                    # sel chunks riding in op A (mult of 4)
WARM_MM = 7                  # dummy matmuls to open the HAM clock gate

_PROGRAM = None
_PROGRAM_KEY = None
_PLAN = None


def _make_plan(n, b2_value, has_b1):
    n = np.asarray(n, dtype=np.int64)
    order = np.argsort(-n, kind="stable")
    loads = [0] * N_CORES
    assign = [[] for _ in range(N_CORES)]
    for b in order:
        cands = [i for i in range(N_CORES) if len(assign[i]) < BAGS]
        c = min(cands, key=lambda i: (loads[i], len(assign[i]), i))
        assign[c].append(int(b))
        loads[c] += int(n[b])
    nch = (max(loads) + CHUNK - 1) // CHUNK
    nch += (-nch) % 4                   # four chunks per (double) PSUM bank
    # fp8 chunk mask: chunk is fp8 iff on EVERY core all its valid
    # columns belong to bags with n >= SMALL_N (padding is always safe)
    fp8 = [USE_FP8] * nch
    for core in range(N_CORES):
        fsc = 0
        found = False
        for b in assign[core]:
            if n[b] < SMALL_N:
                found = True
                break
            fsc += int(n[b])
        if not found:
            fsc = loads[core]
        for c in range(nch):
            if not ((c + 1) * CHUNK <= fsc or c * CHUNK >= loads[core]):
                fp8[c] = False
    return {
        "assign": assign,
        "loads": loads,
        "nch": int(nch),
        "fp8": tuple(bool(v) for v in fp8),
        "n": [int(v) for v in n],
        "b2": float(b2_value),
        "has_b1": bool(has_b1),
    }


def _f8_sched(nf8):
    """DMA ops over the fp8 x chunk slots (beyond op A): ring + count."""
    seq = [("scalar", 16), ("gpsimd", 12), ("scalar", 16), ("sync", 10 ** 9)]
    out = []
    c0 = min(A_CH, nf8)
    for ring, sz in seq:
        if c0 >= nf8:
            break
        sz = min(sz, nf8 - c0)
        out.append((ring, c0, sz))
        c0 += sz
    return out


def _build_program(plan):
    import concourse.bacc as bacc
    import concourse.tile as tile
    from concourse import mybir

    f32 = mybir.dt.float32
    bf16 = mybir.dt.bfloat16
    fp8 = mybir.dt.float8e4
    Alu = mybir.AluOpType
    Act = mybir.ActivationFunctionType
    DR = mybir.MatmulPerfMode.DoubleRow

    NCH = plan["nch"]
    NQUAD = NCH // 4
    has_b1 = plan["has_b1"]
    mask = plan["fp8"]
    f8_slot = {}                # chunk -> slot in megaf8 x region
    b_slot = {}                 # chunk -> slot in megab x region
    for c in range(NCH):
        if mask[c]:
            f8_slot[c] = len(f8_slot)
        else:
            b_slot[c] = len(b_slot)
    NF8, NB = len(f8_slot), len(b_slot)

    nc = bacc.Bacc("TRN2", target_bir_lowering=False, debug=False)

    # megaf8 columns: [w1 (512) | sel-head bytes | x chunks 0..na |
    #                  sel-tail bytes | x chunks na..NF8] -- sel is bf16
    #   data bit-packed into the fp8 tensor and bitcast back on device,
    #   ordered so DMA op A ([0, TAIL0)) delivers w1 + sel head + the
    #   first x chunks in ONE contiguous op.
    selh = 0
    na = min(A_CH, NF8)
    A_X0 = 512
    TAIL0 = A_X0 + na * 256
    XR = TAIL0 + NCH * 2 * BAGS
    MF8C = XR + (NF8 - na) * 256
    if NF8:
        megaf8 = nc.dram_tensor("megaf8", [128, MF8C], fp8,
                                kind="ExternalInput").ap()
    # megab columns: [w1 (512) | bf16 x chunks (NB*256)]
    megab = nc.dram_tensor("megab", [128, 512 + NB * 256], bf16,
                           kind="ExternalInput").ap()
    # aux columns: [w2rep (256) | strip-sum selector (32)]
    aux = nc.dram_tensor("aux", [128, 288], f32, kind="ExternalInput").ap()
    if has_b1:
        b1row = nc.dram_tensor("b1row", [1, D_H], bf16,
                               kind="ExternalInput").ap()
    out = nc.dram_tensor("out", [BAGS, 1], f32, kind="ExternalOutput").ap()

    with tile.TileContext(nc) as tc:
        with (
            tc.tile_pool(name="const", bufs=1) as cpool,
            tc.tile_pool(name="h", bufs=18) as hpool,
            tc.tile_pool(name="z", bufs=3, space="PSUM") as zpool,
            tc.tile_pool(name="po", bufs=1, space="PSUM") as popool,
            tc.tile_pool(name="pr", bufs=1, space="PSUM") as ppool,
        ):
            # PE warmup: the HAM clock gate needs ~3.4us of sustained
            # activity to lift the PE from 1.2 to 2.4 GHz; the PE would
            # otherwise idle during the first DMA and run the first ~6us
            # of real matmuls at half clock. Dummy matmuls on memset
            # data (no DMA dependency) warm it for free.
            wmw = cpool.tile([128, 128], bf16, tag="wmw")
            nc.vector.memset(wmw[:], 0.0)
            wmr = cpool.tile([128, 512], bf16, tag="wmr")
            nc.vector.memset(wmr[:], 0.0)
            wmp = zpool.tile([128, 512], f32, tag="z", name="warm",
                             padded_shape=[128, 1024])
            for _ in range(WARM_MM):
                nc.tensor.matmul(wmp[:], wmw[:], wmr[:],
                                 start=True, stop=True)

            xsb = {}            # fp8 slot -> (tile, col offset)
            # op A on sync: w1f8 + sel head + first fp8 chunks, ONE
            # contiguous op; the sel tail follows as the next sync op
            tA = cpool.tile([128, TAIL0], fp8, tag="opA")
            nc.sync.dma_start(tA[:], megaf8[:, 0:TAIL0])
            w1f8_3 = tA[:, 0:512].rearrange("p (k c) -> p k c", k=2)
            for s in range(na):
                xsb[s] = (tA, A_X0 + s * 256)

            def sel_ap(c):
                if c < selh:
                    a = 512 + c * 2 * BAGS
                    return tA[:, a:a + 2 * BAGS].bitcast(bf16)
                a = (c - selh) * 2 * BAGS
                return tS[:, a:a + 2 * BAGS].bitcast(bf16)

            # sel as the second sync op (first needed ~2us after z starts)
            tS = cpool.tile([128, XR - TAIL0], fp8, tag="selTail")
            nc.sync.dma_start(tS[:], megaf8[:, TAIL0:XR])
            engs = {"sync": nc.sync, "scalar": nc.scalar,
                    "gpsimd": nc.gpsimd}
            for (ring, s0, sz) in _f8_sched(NF8):
                t = cpool.tile([128, sz * 256], fp8, tag=f"xf{s0}")
                engs[ring].dma_start(t[:], megaf8[:, XR + (s0 - na) * 256:
                                                  XR + (s0 - na + sz) * 256])
                for s in range(s0, s0 + sz):
                    xsb[s] = (t, (s - s0) * 256)
            # gpsimd (SWDGE): aux consts + bf16 head (w1 + bf16 chunks)
            auxt = cpool.tile([128, 288], f32, tag="auxt")
            nc.gpsimd.dma_start(auxt[:], aux[:])
            w2t = auxt[:, 0:256]
            stript = auxt[:, 256:288]
            tB = cpool.tile([128, 512 + NB * 256], bf16, tag="opB")
            nc.gpsimd.dma_start(tB[:], megab[:, 0:512 + NB * 256])
            w1at = tB[:, 0:256]
            w1bt = tB[:, 256:512]

            if has_b1:
                b1t = cpool.tile([1, D_H], bf16, tag="b1t")
                nc.gpsimd.dma_start(b1t[:], b1row[:])
                ones1 = cpool.tile([1, 128], bf16, tag="ones1")
                nc.vector.memset(ones1[:], 1.0)

            # per-bag raw sums: 4 col-group strips (or strip 0 only)
            praw = ppool.tile([128, D_H], f32, tag="praw",
                              padded_shape=[128, 512])

            nstrip = 4 if COL_TILE else 1
            strip_chunks = [[c for c in range(NCH) if c % nstrip == j]
                            for j in range(nstrip)]
            first_c = {ch[0] for ch in strip_chunks if ch}
            last_c = {ch[-1] for ch in strip_chunks if ch}

            h_t = [None] * NQUAD

            def emit_chunk(zc, c):
                if mask[c]:
                    t, off = xsb[f8_slot[c]]
                    x3 = t[:, off:off + 256].rearrange(
                        "p (k c) -> p k c", k=2)
                    nc.tensor.matmul(zc, x3, w1f8_3, perf_mode=DR,
                                     start=True, stop=not has_b1)
                else:
                    off = 512 + b_slot[c] * 256
                    nc.tensor.matmul(zc, tB[:, off:off + 128], w1at,
                                     start=True, stop=False)
                    nc.tensor.matmul(zc, tB[:, off + 128:off + 256], w1bt,
                                     start=False, stop=not has_b1)
                if has_b1:
                    nc.tensor.matmul(zc, ones1[:], b1t[:],
                                     start=False, stop=True)

            def emit_quad(q):
                zp = zpool.tile([128, 1024], f32, tag="z", name=f"z_{q}")
                for k in range(4):
                    emit_chunk(zp[:, 256 * k:256 * k + 256], 4 * q + k)
                hp = hpool.tile([128, 1024], bf16, tag="h", name=f"h_{q}")
                if q == NQUAD - 1:
                    # last quad is on the critical tail: split the relu
                    # across both engines
                    nc.scalar.activation(hp[:, 0:512], zp[:, 0:512],
                                         Act.Relu)
                    nc.vector.tensor_scalar(hp[:, 512:1024],
                                            zp[:, 512:1024],
                                            0.0, None, op0=Alu.max)
                elif q % 2 == 0:
                    nc.scalar.activation(hp[:], zp[:], Act.Relu)
                else:
                    nc.vector.tensor_scalar(hp[:], zp[:], 0.0, None,
                                            op0=Alu.max)
                h_t[q] = hp

            def emit_sel_batch(bi):
                for c in range(8 * bi, min(8 * bi + 8, NCH)):
                    j = c % nstrip
                    hp = h_t[c // 4]
                    rhs = hp[:, 256 * (c % 4):256 * (c % 4) + 256]
                    tp = (0, 32 * j) if COL_TILE else None
                    nc.tensor.matmul(praw[32 * j:32 * j + 32, :],
                                     sel_ap(c), rhs,
                                     start=(c in first_c), stop=(c in last_c),
                                     tile_position=tp, skip_group_check=True)

            nbatch = (NCH + 7) // 8
            done_b = 0
            for q in range(NQUAD):
                emit_quad(q)
                b = (q - 3) // 2
                while done_b <= b:
                    emit_sel_batch(done_b)
                    done_b += 1
            while done_b < nbatch:
                emit_sel_batch(done_b)
                done_b += 1

            # ---- finalization: per-strip w2 dot + strip-sum matmul;
            # the 1/n scale and +b2 happen on host (b2=0 typically) ----
            stt = cpool.tile([128, D_H], f32, tag="stt")
            o4 = cpool.tile([128, 1], f32, tag="o4")
            nc.vector.scalar_tensor_tensor(
                stt[:], praw[:], 1.0, w2t,
                op0=Alu.mult, op1=Alu.mult, accum_out=o4[:])
            po = popool.tile([BAGS, 1], f32, tag="po",
                             padded_shape=[128, 512])
            nc.tensor.matmul(po[:], stript, o4[:], start=True, stop=True)
            osb = cpool.tile([BAGS, 1], f32, tag="osb")
            nc.vector.tensor_copy(osb[:], po[:])
            nc.scalar.dma_start(out[:], osb[:])

    nc.compile()
    return nc


def get_program(plan):
    global _PROGRAM, _PROGRAM_KEY
    key = (plan["b2"], plan["nch"], plan["has_b1"], plan["fp8"], COL_TILE)
    if _PROGRAM is None or _PROGRAM_KEY != key:
        _PROGRAM = _build_program(plan)
        _PROGRAM_KEY = key
    return _PROGRAM


def make_in_maps(x, n_instances, W1, b1, W2, b2=None):
    global _PLAN
    x = np.asarray(x, dtype=np.float32)
    n = np.asarray(n_instances, dtype=np.int32)
    W1 = np.asarray(W1, dtype=np.float32)
    b1 = np.asarray(b1, dtype=np.float32).reshape(-1)
    W2 = np.asarray(W2, dtype=np.float32).reshape(-1)
    b2v = 0.0 if b2 is None else float(np.asarray(b2).reshape(-1)[0])
    has_b1 = bool(np.any(b1 != 0.0))
    plan = _make_plan(n, b2v, has_b1)
    _PLAN = plan
    assign, NCH = plan["assign"], plan["nch"]
    mask = np.array(plan["fp8"], dtype=bool)
    COLS = NCH * CHUNK

    w1cat = np.concatenate([W1[0:128, :], W1[128:256, :]], axis=1)
    w1_f8 = w1cat.astype(ml_dtypes.float8_e4m3)
    w1_bf = w1cat.astype(ml_dtypes.bfloat16)
    auxm = np.zeros((128, 288), dtype=np.float32)
    auxm[:, 0:256] = W2.reshape(1, D_H)
    for j in range(4 if COL_TILE else 1):
        auxm[32 * j + np.arange(32), 256 + np.arange(32)] = 1.0

    in_maps = []
    for c in range(N_CORES):
        bags = assign[c]
        ns = np.array([n[b] for b in bags], dtype=np.int64)
        starts = np.concatenate([[0], np.cumsum(ns)])
        total = int(starts[-1])
        # X: [256 din, COLS] valid instance columns, bag-major
        X = np.zeros((D_IN, COLS), dtype=np.float32)
        for s, b in enumerate(bags):
            X[:, starts[s]:starts[s + 1]] = x[b, :ns[s], :].T
        # chunk-major x: [128, chunk, (half, col)]
        Xr = X.reshape(D_IN, NCH, CHUNK)
        xa = np.empty((128, NCH, 256), dtype=np.float32)
        xa[:, :, 0:128] = Xr[0:128]
        xa[:, :, 128:256] = Xr[128:256]
        # selector: sel[(col %128), chunk*BAGS + slot] = 1 for valid cols
        sel = np.zeros((128, NCH, BAGS), dtype=np.float32)
        cols_idx = np.arange(total)
        slot_of = np.repeat(np.arange(BAGS), ns)
        sel[cols_idx % CHUNK, cols_idx // CHUNK, slot_of] = 1.0
        nf8 = int(mask.sum())
        na = min(A_CH, nf8)
        selb = np.ascontiguousarray(
            sel.reshape(128, NCH * BAGS).astype(ml_dtypes.bfloat16)
        ).view(ml_dtypes.float8_e4m3)
        xf8 = xa[:, mask, :].reshape(128, nf8 * 256).astype(
            ml_dtypes.float8_e4m3)
        im = {}
        im["megaf8"] = np.concatenate([
            w1_f8,
            xf8[:, :na * 256],
            selb,
            xf8[:, na * 256:],
        ], axis=1)
        im["megab"] = np.concatenate([
            w1_bf,
            xa[:, ~mask, :].reshape(128, (NCH - nf8) * 256)
            .astype(ml_dtypes.bfloat16),
        ], axis=1)
        im["aux"] = auxm
        if has_b1:
            im["b1row"] = b1.reshape(1, D_H).astype(ml_dtypes.bfloat16)
        in_maps.append(im)
    return in_maps


def run_spmd(in_maps, b2_value=0.0, trace=False, **kwargs):
    from concourse import bass_utils
    if trace:
        # no S3 in this environment; keep trace artifacts local
        bass_utils.upload_artifacts = lambda tmpdir: tmpdir
    nc = get_program(_PLAN)
    try:
        return bass_utils.run_bass_kernel_spmd(
            nc, in_maps, core_ids=list(range(N_CORES)), trace=trace,
            **kwargs)
    except Exception:
        # rare transient NRT_EXEC_UNIT_UNRECOVERABLE on the first launch
        # of a fresh NEFF; one retry usually succeeds
        import time
        time.sleep(2.0)
        return bass_utils.run_bass_kernel_spmd(
            nc, in_maps, core_ids=list(range(N_CORES)), trace=trace,
            **kwargs)


def kernel(x, n_instances, W1, b1, W2, b2):
    b2_value = float(np.asarray(b2).reshape(-1)[0])
    in_maps = make_in_maps(x, n_instances, W1, b1, W2, b2)
    res = run_spmd(in_maps, b2_value=b2_value)
    n = np.asarray(n_instances).reshape(-1)
    out = np.empty((B, 1), dtype=np.float32)
    for c in range(N_CORES):
        ps = np.asarray(res.results[c]["out"], dtype=np.float32).reshape(-1)
        for s, b in enumerate(_PLAN["assign"][c]):
            out[b, 0] = ps[s] / float(n[b]) + b2_value
    return out


# revision 34
# speedup vs baseline: 1.0381x; 1.0381x over previous
"""Trainium2 Bass kernel for nn_BagModel_3d (segment_reduce).

Computation (per bag b):
  out[b] = (1/n_b) * sum_{i < n_b} relu(x[b, i, :] @ W1 + b1) @ W2 + b2

Strategy (v3 -- instances-on-partitions dataflow, fp8 DoubleRow z):

- Data-parallel over bags: LPT assigns exactly 32 bags per core
  (descending n, so small bags land at the tail of each core's column
  space). Valid instance columns are laid out contiguously and split
  into 128-column CHUNKs.
- Per chunk, the x block is the matmul STATIONARY operand and W1
  streams, so z lands as [cols(part), dh(free)] in PSUM. Chunks whose
  bags all have n >= SMALL_N use fp8-e4m3 x/W1 with perf_mode=DoubleRow
  (contraction 256 in ONE matmul; quantization noise averages down as
  1/sqrt(n) in the bag mean); chunks touching smaller bags use bf16
  with two matmuls. Two chunks share one PSUM bank; one relu per bank
  ([128, 512], ScalarE activation / VectorE tensor_scalar alternating)
  produces h in bf16 -- no per-bag drains, no on-device casts.
- The ragged per-bag sum is a 0/1 selector matmul: praw[bag, dh] +=
  sel_c^T @ h_c, accumulated in PSUM across all chunks; bag boundaries
  are data, padding columns contribute exactly 0, and chunks are
  striped over the 4 PE column-groups (tile_position) so 4 selector
  matmuls run concurrently in the array.
- Finalization: o4 = sum_dh praw*W2 (one fused DVE op with accum),
  strip-sum via a tiny matmul, then out = o4/n + b2.
- DMA: each HWDGE ring serializes its ops with a ~2.2us completion
  tax, so ops are few and large: op A (sync) carries w1+first chunks
  in ONE op to minimize PE start latency; the sel matrix is split so
  its head rides early on the scalar ring; slow-start SWDGE (gpsimd)
  carries late-needed data (aux consts, bf16 tail chunks).
- b1 (zero in this workload) is handled generally via a rank-1 k=1
  matmul added into z before relu, emitted only when b1 != 0.
"""
import sys
import numpy as np

sys.path.insert(0, '/opt/trn_rl_repo')

import ml_dtypes

B, N_MAX, D_IN, D_H = 256, 512, 256, 256
N_CORES = 8
BAGS = B // N_CORES          # 32 bags per core
CHUNK = 128                  # instance columns per matmul chunk
COL_TILE = True              # stripe selector matmuls over PE col-groups
SEL_AT_END = True            # emit all selector MMs after the z stream
USE_FP8 = True               # fp8 DoubleRow z for big-bag chunks
SMALL_N = 16                 # bags below this stay bf16
A_CH = 8                     # x chunks bundled with w1+sel-head in op A
SELH = 16                    # sel chunks riding in op A (mult of 4)
WARM_MM = 7                  # dummy matmuls to open the HAM clock gate

_PROGRAM = None
_PROGRAM_KEY = None
_PLAN = None


def _make_plan(n, b2_value, has_b1):
    n = np.asarray(n, dtype=np.int64)
    order = np.argsort(-n, kind="stable")
    loads = [0] * N_CORES
    assign = [[] for _ in range(N_CORES)]
    for b in order:
        cands = [i for i in range(N_CORES) if len(assign[i]) < BAGS]
        c = min(cands, key=lambda i: (loads[i], len(assign[i]), i))
        assign[c].append(int(b))
        loads[c] += int(n[b])
    nch = (max(loads) + CHUNK - 1) // CHUNK
    nch += (-nch) % 4                   # four chunks per (double) PSUM bank
    # fp8 chunk mask: chunk is fp8 iff on EVERY core all its valid
    # columns belong to bags with n >= SMALL_N (padding is always safe)
    fp8 = [USE_FP8] * nch
    for core in range(N_CORES):
        fsc = 0
        found = False
        for b in assign[core]:
            if n[b] < SMALL_N:
                found = True
                break
            fsc += int(n[b])
        if not found:
            fsc = loads[core]
        for c in range(nch):
            if not ((c + 1) * CHUNK <= fsc or c * CHUNK >= loads[core]):
                fp8[c] = False
    return {
        "assign": assign,
        "loads": loads,
        "nch": int(nch),
        "fp8": tuple(bool(v) for v in fp8),
        "n": [int(v) for v in n],
        "b2": float(b2_value),
        "has_b1": bool(has_b1),
    }


def _f8_sched(nf8):
    """DMA ops over the fp8 x chunk slots (beyond op A): ring + count."""
    seq = [("scalar", 16), ("gpsimd", 12), ("scalar", 16), ("sync", 10 ** 9)]
    out = []
    c0 = min(A_CH, nf8)
    for ring, sz in seq:
        if c0 >= nf8:
            break
        sz = min(sz, nf8 - c0)
        out.append((ring, c0, sz))
        c0 += sz
    return out


def _build_program(plan):
    import concourse.bacc as bacc
    import concourse.tile as tile
    from concourse import mybir

    f32 = mybir.dt.float32
    bf16 = mybir.dt.bfloat16
    fp8 = mybir.dt.float8e4
    Alu = mybir.AluOpType
    Act = mybir.ActivationFunctionType
    DR = mybir.MatmulPerfMode.DoubleRow

    NCH = plan["nch"]
    NQUAD = NCH // 4
    has_b1 = plan["has_b1"]
    mask = plan["fp8"]
    f8_slot = {}                # chunk -> slot in megaf8 x region
    b_slot = {}                 # chunk -> slot in megab x region
    for c in range(NCH):
        if mask[c]:
            f8_slot[c] = len(f8_slot)
        else:
            b_slot[c] = len(b_slot)
    NF8, NB = len(f8_slot), len(b_slot)

    nc = bacc.Bacc("TRN2", target_bir_lowering=False, debug=False)

    # megaf8 columns: [w1 (512) | sel-head bytes | x chunks 0..na |
    #                  sel-tail bytes | x chunks na..NF8] -- sel is bf16
    #   data bit-packed into the fp8 tensor and bitcast back on device,
    #   ordered so DMA op A ([0, TAIL0)) delivers w1 + sel head + the
    #   first x chunks in ONE contiguous op.
    selh = min(SELH, NCH)
    na = min(A_CH, NF8)
    A_X0 = 512 + selh * 2 * BAGS
    TAIL0 = A_X0 + na * 256
    XR = TAIL0 + (NCH - selh) * 2 * BAGS
    MF8C = XR + (NF8 - na) * 256
    if NF8:
        megaf8 = nc.dram_tensor("megaf8", [128, MF8C], fp8,
                                kind="ExternalInput").ap()
    # megab columns: [w1 (512) | bf16 x chunks (NB*256)]
    megab = nc.dram_tensor("megab", [128, 512 + NB * 256], bf16,
                           kind="ExternalInput").ap()
    # aux columns: [w2rep (256) | strip-sum selector (32)]
    aux = nc.dram_tensor("aux", [128, 288], f32, kind="ExternalInput").ap()
    if has_b1:
        b1row = nc.dram_tensor("b1row", [1, D_H], bf16,
                               kind="ExternalInput").ap()
    out = nc.dram_tensor("out", [BAGS, 1], f32, kind="ExternalOutput").ap()

    with tile.TileContext(nc) as tc:
        with (
            tc.tile_pool(name="const", bufs=1) as cpool,
            tc.tile_pool(name="h", bufs=18) as hpool,
            tc.tile_pool(name="z", bufs=3, space="PSUM") as zpool,
            tc.tile_pool(name="po", bufs=1, space="PSUM") as popool,
            tc.tile_pool(name="pr", bufs=1, space="PSUM") as ppool,
        ):
            # PE warmup: the HAM clock gate needs ~3.4us of sustained
            # activity to lift the PE from 1.2 to 2.4 GHz; the PE would
            # otherwise idle during the first DMA and run the first ~6us
            # of real matmuls at half clock. Dummy matmuls on memset
            # data (no DMA dependency) warm it for free.
            wmw = cpool.tile([128, 128], bf16, tag="wmw")
            nc.vector.memset(wmw[:], 0.0)
            wmr = cpool.tile([128, 512], bf16, tag="wmr")
            nc.vector.memset(wmr[:], 0.0)
            wmp = zpool.tile([128, 512], f32, tag="z", name="warm",
                             padded_shape=[128, 1024])
            for _ in range(WARM_MM):
                nc.tensor.matmul(wmp[:], wmw[:], wmr[:],
                                 start=True, stop=True)

            xsb = {}            # fp8 slot -> (tile, col offset)
            # op A on sync: w1f8 + sel head + first fp8 chunks, ONE
            # contiguous op; the sel tail follows as the next sync op
            tA = cpool.tile([128, TAIL0], fp8, tag="opA")
            nc.sync.dma_start(tA[:], megaf8[:, 0:TAIL0])
            w1f8_3 = tA[:, 0:512].rearrange("p (k c) -> p k c", k=2)
            for s in range(na):
                xsb[s] = (tA, A_X0 + s * 256)

            def sel_ap(c):
                if c < selh:
                    a = 512 + c * 2 * BAGS
                    return tA[:, a:a + 2 * BAGS].bitcast(bf16)
                a = (c - selh) * 2 * BAGS
                return tS[:, a:a + 2 * BAGS].bitcast(bf16)

            engs = {"sync": nc.sync, "scalar": nc.scalar,
                    "gpsimd": nc.gpsimd}
            for (ring, s0, sz) in _f8_sched(NF8):
                t = cpool.tile([128, sz * 256], fp8, tag=f"xf{s0}")
                engs[ring].dma_start(t[:], megaf8[:, XR + (s0 - na) * 256:
                                                  XR + (s0 - na + sz) * 256])
                for s in range(s0, s0 + sz):
                    xsb[s] = (t, (s - s0) * 256)
            # sel tail rides sync after its x ops (only needed late)
            tS = None
            if XR > TAIL0:
                tS = cpool.tile([128, XR - TAIL0], fp8, tag="selTail")
                nc.sync.dma_start(tS[:], megaf8[:, TAIL0:XR])

            # gpsimd (SWDGE): aux consts + bf16 head (w1 + bf16 chunks)
            auxt = cpool.tile([128, 288], f32, tag="auxt")
            nc.gpsimd.dma_start(auxt[:], aux[:])
            w2t = auxt[:, 0:256]
            stript = auxt[:, 256:288]
            tB = cpool.tile([128, 512 + NB * 256], bf16, tag="opB")
            nc.gpsimd.dma_start(tB[:], megab[:, 0:512 + NB * 256])
            w1at = tB[:, 0:256]
            w1bt = tB[:, 256:512]

            if has_b1:
                b1t = cpool.tile([1, D_H], bf16, tag="b1t")
                nc.gpsimd.dma_start(b1t[:], b1row[:])
                ones1 = cpool.tile([1, 128], bf16, tag="ones1")
                nc.vector.memset(ones1[:], 1.0)

            # per-bag raw sums: 4 col-group strips (or strip 0 only)
            praw = ppool.tile([128, D_H], f32, tag="praw",
                              padded_shape=[128, 512])

            nstrip = 4 if COL_TILE else 1
            strip_chunks = [[c for c in range(NCH) if c % nstrip == j]
                            for j in range(nstrip)]
            first_c = {ch[0] for ch in strip_chunks if ch}
            last_c = {ch[-1] for ch in strip_chunks if ch}

            h_t = [None] * NQUAD

            def emit_chunk(zc, c):
                if mask[c]:
                    t, off = xsb[f8_slot[c]]
                    x3 = t[:, off:off + 256].rearrange(
                        "p (k c) -> p k c", k=2)
                    nc.tensor.matmul(zc, x3, w1f8_3, perf_mode=DR,
                                     start=True, stop=not has_b1)
                else:
                    off = 512 + b_slot[c] * 256
                    nc.tensor.matmul(zc, tB[:, off:off + 128], w1at,
                                     start=True, stop=False)
                    nc.tensor.matmul(zc, tB[:, off + 128:off + 256], w1bt,
                                     start=False, stop=not has_b1)
                if has_b1:
                    nc.tensor.matmul(zc, ones1[:], b1t[:],
                                     start=False, stop=True)

            def emit_quad(q):
                zp = zpool.tile([128, 1024], f32, tag="z", name=f"z_{q}")
                for k in range(4):
                    emit_chunk(zp[:, 256 * k:256 * k + 256], 4 * q + k)
                hp = hpool.tile([128, 1024], bf16, tag="h", name=f"h_{q}")
                if q == NQUAD - 1:
                    # last quad is on the critical tail: split the relu
                    # across both engines
                    nc.scalar.activation(hp[:, 0:512], zp[:, 0:512],
                                         Act.Relu)
                    nc.vector.tensor_scalar(hp[:, 512:1024],
                                            zp[:, 512:1024],
                                            0.0, None, op0=Alu.max)
                elif q % 2 == 0:
                    nc.scalar.activation(hp[:], zp[:], Act.Relu)
                else:
                    nc.vector.tensor_scalar(hp[:], zp[:], 0.0, None,
                                            op0=Alu.max)
                h_t[q] = hp

            def emit_sel_batch(bi):
                for c in range(8 * bi, min(8 * bi + 8, NCH)):
                    j = c % nstrip
                    hp = h_t[c // 4]
                    rhs = hp[:, 256 * (c % 4):256 * (c % 4) + 256]
                    tp = (0, 32 * j) if COL_TILE else None
                    nc.tensor.matmul(praw[32 * j:32 * j + 32, :],
                                     sel_ap(c), rhs,
                                     start=(c in first_c), stop=(c in last_c),
                                     tile_position=tp, skip_group_check=True)

            nbatch = (NCH + 7) // 8
            done_b = 0
            for q in range(NQUAD):
                emit_quad(q)
                b = (q - 3) // 2
                while done_b <= b:
                    emit_sel_batch(done_b)
                    done_b += 1
            while done_b < nbatch:
                emit_sel_batch(done_b)
                done_b += 1

            # ---- finalization: per-strip w2 dot + strip-sum matmul;
            # the 1/n scale and +b2 happen on host (b2=0 typically) ----
            stt = cpool.tile([128, D_H], f32, tag="stt")
            o4 = cpool.tile([128, 1], f32, tag="o4")
            nc.vector.scalar_tensor_tensor(
                stt[:], praw[:], 1.0, w2t,
                op0=Alu.mult, op1=Alu.mult, accum_out=o4[:])
            po = popool.tile([BAGS, 1], f32, tag="po",
                             padded_shape=[128, 512])
            nc.tensor.matmul(po[:], stript, o4[:], start=True, stop=True)
            osb = cpool.tile([BAGS, 1], f32, tag="osb")
            nc.vector.tensor_copy(osb[:], po[:])
            nc.scalar.dma_start(out[:], osb[:])

    nc.compile()
    return nc


def get_program(plan):
    global _PROGRAM, _PROGRAM_KEY
    key = (plan["b2"], plan["nch"], plan["has_b1"], plan["fp8"], COL_TILE)
    if _PROGRAM is None or _PROGRAM_KEY != key:
        _PROGRAM = _build_program(plan)
        _PROGRAM_KEY = key
    return _PROGRAM


def make_in_maps(x, n_instances, W1, b1, W2, b2=None):
    global _PLAN
    x = np.asarray(x, dtype=np.float32)
    n = np.asarray(n_instances, dtype=np.int32)
    W1 = np.asarray(W1, dtype=np.float32)
    b1 = np.asarray(b1, dtype=np.float32).reshape(-1)
    W2 = np.asarray(W2, dtype=np.float32).reshape(-1)
    b2v = 0.0 if b2 is None else float(np.asarray(b2).reshape(-1)[0])
    has_b1 = bool(np.any(b1 != 0.0))
    plan = _make_plan(n, b2v, has_b1)
    _PLAN = plan
    assign, NCH = plan["assign"], plan["nch"]
    mask = np.array(plan["fp8"], dtype=bool)
    COLS = NCH * CHUNK

    w1cat = np.concatenate([W1[0:128, :], W1[128:256, :]], axis=1)
    w1_f8 = w1cat.astype(ml_dtypes.float8_e4m3)
    w1_bf = w1cat.astype(ml_dtypes.bfloat16)
    auxm = np.zeros((128, 288), dtype=np.float32)
    auxm[:, 0:256] = W2.reshape(1, D_H)
    for j in range(4 if COL_TILE else 1):
        auxm[32 * j + np.arange(32), 256 + np.arange(32)] = 1.0

    in_maps = []
    for c in range(N_CORES):
        bags = assign[c]
        ns = np.array([n[b] for b in bags], dtype=np.int64)
        starts = np.concatenate([[0], np.cumsum(ns)])
        total = int(starts[-1])
        # X: [256 din, COLS] valid instance columns, bag-major
        X = np.zeros((D_IN, COLS), dtype=np.float32)
        for s, b in enumerate(bags):
            X[:, starts[s]:starts[s + 1]] = x[b, :ns[s], :].T
        # chunk-major x: [128, chunk, (half, col)]
        Xr = X.reshape(D_IN, NCH, CHUNK)
        xa = np.empty((128, NCH, 256), dtype=np.float32)
        xa[:, :, 0:128] = Xr[0:128]
        xa[:, :, 128:256] = Xr[128:256]
        # selector: sel[(col %128), chunk*BAGS + slot] = 1 for valid cols
        sel = np.zeros((128, NCH, BAGS), dtype=np.float32)
        cols_idx = np.arange(total)
        slot_of = np.repeat(np.arange(BAGS), ns)
        sel[cols_idx % CHUNK, cols_idx // CHUNK, slot_of] = 1.0
        nf8 = int(mask.sum())
        selh = min(SELH, NCH)
        na = min(A_CH, nf8)
        selb = np.ascontiguousarray(
            sel.reshape(128, NCH * BAGS).astype(ml_dtypes.bfloat16)
        ).view(ml_dtypes.float8_e4m3)
        xf8 = xa[:, mask, :].reshape(128, nf8 * 256).astype(
            ml_dtypes.float8_e4m3)
        im = {}
        im["megaf8"] = np.concatenate([
            w1_f8,
            selb[:, :selh * 2 * BAGS],
            xf8[:, :na * 256],
            selb[:, selh * 2 * BAGS:],
            xf8[:, na * 256:],
        ], axis=1)
        im["megab"] = np.concatenate([
            w1_bf,
            xa[:, ~mask, :].reshape(128, (NCH - nf8) * 256)
            .astype(ml_dtypes.bfloat16),
        ], axis=1)
        im["aux"] = auxm
        if has_b1:
            im["b1row"] = b1.reshape(1, D_H).astype(ml_dtypes.bfloat16)
        in_maps.append(im)
    return in_maps


def run_spmd(in_maps, b2_value=0.0, trace=False, **kwargs):
    from concourse import bass_utils
    if trace:
        # no S3 in this environment; keep trace artifacts local
        bass_utils.upload_artifacts = lambda tmpdir: tmpdir
    nc = get_program(_PLAN)
    try:
        return bass_utils.run_bass_kernel_spmd(
            nc, in_maps, core_ids=list(range(N_CORES)), trace=trace,
            **kwargs)
    except Exception:
        # rare transient NRT_EXEC_UNIT_UNRECOVERABLE on the first launch
        # of a fresh NEFF; one retry usually succeeds
        import time
        time.sleep(2.0)
        return bass_utils.run_bass_kernel_spmd(
            nc, in_maps, core_ids=list(range(N_CORES)), trace=trace,
            **kwargs)


def kernel(x, n_instances, W1, b1, W2, b2):
    b2_value = float(np.asarray(b2).reshape(-1)[0])
    in_maps = make_in_maps(x, n_instances, W1, b1, W2, b2)
    res = run_spmd(in_maps, b2_value=b2_value)
    n = np.asarray(n_instances).reshape(-1)
    out = np.empty((B, 1), dtype=np.float32)
    for c in range(N_CORES):
        ps = np.asarray(res.results[c]["out"], dtype=np.float32).reshape(-1)
        for s, b in enumerate(_PLAN["assign"][c]):
            out[b, 0] = ps[s] / float(n[b]) + b2_value
    return out
